# revision 1
# baseline (speedup 1.0000x reference)
"""EulerFormer kernel for Trainium2 (8 NeuronCores, data-parallel over batch).

Math (per batch):
    scores = mean_l v[l, :]                          # [D]
    P      = neuralsort-softmax(scores)              # [D, D]
    vs     = v @ P.T                                 # [L, D]
    r, p   = vs[..., ::2], vs[..., 1::2]
    lam    = sqrt(r^2 + p^2 + eps)
    theta  = atan2(p, r) * delta  (== 2*atan(p/(lam+r)) * delta)
    out    = interleave(lam' * cos(theta), lam' * sin(theta)),
             lam' = lam * exp(clip(log_scale, -5, 5))

Sharding: batch 16 -> 2 per core across 8 cores. Per core:
  phase 1 (both batches): load v, PE-transpose to vT (f32r rounded on the
    PSUM->SBUF evac, which also yields scores partial sums), build P, PT.
  phase 2 (both batches): f32r einsum on PE; elementwise epilogue on ACT/DVE.
  ACT table sets are grouped program-wide: [exp (P)] -> [sqrt] -> [trig],
  enforced with scheduling-order edges, so only ~3 table loads happen.
"""

import sys

sys.path.insert(0, "/opt/trn_rl_repo")

import numpy as np

import concourse.bacc as bacc
import concourse.mybir as mybir
import concourse.tile as tile
from concourse.bass_utils import run_bass_kernel_spmd
from concourse.tile_rust import add_dep_helper

import concourse.dve_ops as dve_ops
from concourse.dve_spec import Spec, Src0, Src1, C0, sq, _has_src1, lower as dve_lower
from concourse.dve_uop import DveOpSpec


def _np_sqsum_ref(in0, in1, c0, c1, c2):
    return in0.astype(np.float32) ** 2 + in1.astype(np.float32) ** 2 + c0


def _register_sqsum():
    """out = in0^2 + in1^2 + s0, one DVE pass (replaces square+square+add)."""
    name = "TENSOR_SQSUM_ANT"
    if name in dve_ops._SUB_OPCODE_FOR_NAME:
        return next(op for op in dve_ops.OPS if op.name == name)
    spec = Spec(body=sq(Src0) + sq(Src1) + C0, reference=_np_sqsum_ref)
    row = max(dve_ops._SUB_OPCODE_FOR_NAME.values()) + 1
    assert row < 0x20
    op = dve_ops.DveOp(name, spec, subdim=False, uops_sha={})
    dve_ops._SUB_OPCODE_FOR_NAME[name] = row
    dve_ops.OPS.append(op)
    dve_ops.CUSTOM_DVE_SPECS[name] = spec
    for ver in ("v3", "v4"):
        try:
            compiled = DveOpSpec(
                name=name, opcode=row, uops=dve_lower(spec, ver=ver),
                rd1_en=_has_src1(spec),
            )
            op.uops_sha[ver] = compiled.sha(ver)
        except Exception:
            pass
    return op


SQSUM_OP = _register_sqsum()

F32 = mybir.dt.float32
F32R = mybir.dt.float32r
AF = mybir.ActivationFunctionType
ALU = mybir.AluOpType
AX = mybir.AxisListType

B, L, D = 16, 4096, 256
NCORES = 8
B_PER = B // NCORES  # 2
DH = D // 2  # 128 pairs per l-row
LT = 128
NLT = L // LT  # 32 l-tiles
TAU = 1.0
EPS = 1e-6
HALF_PI = float(np.pi / 2)

G8 = 8  # l-tiles per load/transpose group
NG8 = NLT // G8  # 4
G4 = 4  # l-tiles per einsum psum group
NG4 = NLT // G4  # 8
PAIR_COLS = NLT * DH  # 4096 pair columns per batch
VS_COLS = NLT * D  # 8192 vs-slab columns per batch
NCH = 4  # epilogue chunks per batch
CHP = PAIR_COLS // NCH  # 2048 pair cols per chunk


def _reg_consts(nc, vals):
    for val in vals:
        val = float(val)
        if (F32, val) in nc.const_aps.aps:
            continue
        t = nc.alloc_sbuf_tensor(f"const-float32-{val}", [128, 1], F32)
        nc.gpsimd.memset(t.ap(), val)
        nc.const_aps.aps[(F32, val)] = t.ap()
    nc.all_engine_barrier()


def build_program(two_delta, use_esc, use_delta_vec, reps=1, global_sets=False):
    nc = bacc.Bacc("TRN2", target_bir_lowering=False, debug=False)
    _reg_consts(nc, [HALF_PI])

    v_d = nc.dram_tensor("v", [B_PER, L, D], F32, kind="ExternalInput").ap()
    ident_d = nc.dram_tensor("ident", [128, 128], F32, kind="ExternalInput").ap()
    scal_d = nc.dram_tensor("scalecol", [128, 2], F32, kind="ExternalInput").ap()
    if use_esc:
        esc_d = nc.dram_tensor(
            "esc_rep", [128, PAIR_COLS], F32, kind="ExternalInput"
        ).ap()
    if use_delta_vec:
        d2_d = nc.dram_tensor(
            "delta2_rep", [128, PAIR_COLS], F32, kind="ExternalInput"
        ).ap()
    out_d = nc.dram_tensor("out", [B_PER, L, D], F32, kind="ExternalOutput").ap()

    with tile.TileContext(nc) as tc:
        with (
            tc.tile_pool(name="aux", bufs=1) as aux,
            tc.tile_pool(name="vload", bufs=2) as vload,
            tc.tile_pool(name="vt", bufs=1) as vtp,
            tc.tile_pool(name="vs", bufs=1) as vsp,
            tc.tile_pool(name="pair", bufs=3) as pairp,
            tc.tile_pool(name="pbuild", bufs=2) as pb,
            tc.tile_pool(name="ptr", bufs=1) as ptrp,
            tc.tile_pool(name="psT", bufs=2, space="PSUM") as psT,
            tc.tile_pool(name="psMM", bufs=2, space="PSUM") as psMM,
        ):
            ident = aux.tile([128, 128], F32, tag="ident", name="ident")
            nc.sync.dma_start(ident[:], ident_d)
            scal = aux.tile([128, 2], F32, tag="scal", name="scal")
            nc.sync.dma_start(scal[:], scal_d)
            ones_row = aux.tile([1, 128], F32, tag="ones", name="ones")
            nc.gpsimd.memset(ones_row[:], 1.0)
            if use_esc:
                esc_t = aux.tile([128, PAIR_COLS], F32, tag="esc", name="esc")
                nc.sync.dma_start(esc_t[:], esc_d)
            if use_delta_vec:
                d2_t = aux.tile([128, PAIR_COLS], F32, tag="d2", name="d2")
                nc.sync.dma_start(d2_t[:], d2_d)

            vs_t = [
                vsp.tile([128, VS_COLS], F32, tag=f"vs{bi}", name=f"vs{bi}")
                for bi in range(B_PER)
            ]
            pt_t = [
                [
                    ptrp.tile([128, D], F32R, tag=f"pt{bi}{ch}", name=f"pt{bi}{ch}")
                    for ch in range(2)
                ]
                for bi in range(B_PER)
            ]

            rowrep_n = [0]

            def rowrep(colpair_tile, scale, tagbase):
                """[128, 2] per-chunk columns -> [128, 256] row-replicated."""
                ps1 = psMM.tile([128, G4 * D], F32, tag="psmm", name="psmm")
                nc.tensor.transpose(ps1[0:1, 0:128], colpair_tile[:, 0:1], ident[:])
                nc.tensor.transpose(ps1[0:1, 128:256], colpair_tile[:, 1:2], ident[:])
                rowrep_n[0] += 1
                flat = pb.tile([1, D], F32, tag=f"{tagbase}f", name=f"{tagbase}f{rowrep_n[0]}")
                nc.scalar.activation(flat[:], ps1[0:1, 0:D], AF.Identity, scale=scale)
                ps2 = psMM.tile([128, G4 * D], F32, tag="psmm", name="psmm")
                nc.tensor.matmul(
                    ps2[:, 0:D], ones_row[:], flat[:], start=True, stop=True
                )
                rep = pb.tile([128, D], F32, tag=f"{tagbase}r", name=f"{tagbase}r{rowrep_n[0]}")
                nc.scalar.activation(rep[:], ps2[:, 0:D], AF.Identity)
                return rep

            unit_marks = []

            for rep in range(reps):
                vt_all = []
                sp_t = []
                # ===== per batch: load/transpose/scores/P then einsum+D0 ====
                for bi in range(B_PER):
                    exp_last = [None]
                    sqrt_first, sqrt_last = [None], [None]
                    trig_first = [None]
                    unit_marks.append((exp_last, sqrt_first, sqrt_last, trig_first))
                    vt = [
                        vtp.tile([128, L], F32R, tag=f"vt{ch}", name=f"vt{bi}{ch}")
                        for ch in range(2)
                    ]
                    vt_all.append(vt)
                    partials = pb.tile(
                        [128, 2 * NG8], F32, tag="part", name=f"part{bi}"
                    )
                    for g in range(NG8):
                        lv = vload.tile([128, G8 * D], F32, tag="lv", name="lv")
                        src = v_d[bi, g * G8 * LT : (g + 1) * G8 * LT, :].rearrange(
                            "(t p) j -> p t j", p=128
                        )
                        nc.sync.dma_start(
                            lv[:].rearrange("p (t j) -> p t j", j=D), src
                        )
                        for ch in range(2):
                            pst = psT.tile(
                                [128, G8 * 128], F32, tag="psT", name="psT"
                            )
                            for t in range(G8):
                                nc.tensor.transpose(
                                    pst[:, t * 128 : (t + 1) * 128],
                                    lv[:, t * D + ch * 128 : t * D + (ch + 1) * 128],
                                    ident[:],
                                )
                            dst = vt[ch][:, g * G8 * LT : (g + 1) * G8 * LT]
                            acc = partials[:, ch * NG8 + g : ch * NG8 + g + 1]
                            nc.scalar.activation(
                                dst, pst[:], AF.Identity, accum_out=acc
                            )

                    # ----- scores + P build --------------------------------
                    ssum = pb.tile([128, 2], F32, tag="ssum", name=f"ssum{bi}")
                    for ch in range(2):
                        nc.vector.tensor_reduce(
                            ssum[:, ch : ch + 1],
                            partials[:, ch * NG8 : (ch + 1) * NG8],
                            axis=AX.X,
                            op=ALU.add,
                        )
                    srow = rowrep(ssum, 1.0, "sr")
                    bsum = pb.tile([128, 2], F32, tag="bsum", name=f"bsum{bi}")
                    scratch = pb.tile([128, D], F32, tag="scr", name=f"scr{bi}")
                    for ch in range(2):
                        nc.vector.tensor_scalar(
                            scratch[:], srow[:], ssum[:, ch : ch + 1], None,
                            ALU.subtract,
                        )
                        nc.scalar.activation(
                            scratch[:], scratch[:], AF.Abs,
                            accum_out=bsum[:, ch : ch + 1],
                        )
                    brow = rowrep(bsum, 1.0 / (L * TAU), "br")

                    expp = [None, None]
                    for ch in range(2):
                        pmax = pb.tile(
                            [128, D], F32, tag=f"pmax{ch}", name=f"pmax{bi}{ch}"
                        )
                        nc.vector.scalar_tensor_tensor(
                            pmax[:], srow[:], scal[:, ch : ch + 1], brow[:],
                            ALU.mult, ALU.subtract,
                        )
                        negm = pb.tile(
                            [128, 1], F32, tag=f"negm{ch}", name=f"negm{bi}{ch}"
                        )
                        nc.vector.tensor_reduce(
                            negm[:], pmax[:], axis=AX.X, op=ALU.max, negate=True
                        )
                        rowsum = pb.tile(
                            [128, 1], F32, tag=f"rs{ch}", name=f"rs{bi}{ch}"
                        )
                        expp[ch] = pb.tile(
                            [128, D], F32, tag=f"expp{ch}", name=f"expp{bi}{ch}"
                        )
                        e_ins = nc.scalar.activation(
                            expp[ch][:], pmax[:], AF.Exp,
                            bias=negm[:], accum_out=rowsum[:],
                        )
                        exp_last[0] = e_ins
                        rinv = pb.tile(
                            [128, 1], F32, tag=f"rinv{ch}", name=f"rinv{bi}{ch}"
                        )
                        rscr = pb.tile(
                            [128, 1], F32, tag=f"rscr{ch}", name=f"rscr{bi}{ch}"
                        )
                        nc.vector.reciprocal_approx_accurate(
                            out=rinv[:], in_=rowsum[:], scratch=rscr[:]
                        )
                        nc.vector.tensor_scalar(
                            expp[ch][:], expp[ch][:], rinv[:], None, ALU.mult
                        )
                    for ech in range(2):
                        ps_pt = psMM.tile([128, G4 * D], F32, tag="psmm", name="psmm")
                        for dch in range(2):
                            nc.tensor.transpose(
                                ps_pt[:, dch * 128 : (dch + 1) * 128],
                                expp[dch][:, ech * 128 : (ech + 1) * 128],
                                ident[:],
                            )
                        nc.vector.tensor_copy(pt_t[bi][ech][:], ps_pt[:, 0:D])

                    # -------- einsum + D0 + s' (this batch) ---------------
                    sp = pairp.tile(
                        [128, PAIR_COLS], F32, tag=f"sp{bi}", name=f"sp{bi}", bufs=1
                    )
                    sp_t.append(sp)
                    for g in range(NG4):
                        ps = psMM.tile([128, G4 * D], F32, tag="psmm", name="psmm")
                        for t in range(G4):
                            lt = g * G4 + t
                            for ch in range(2):
                                nc.tensor.matmul(
                                    ps[:, t * D : (t + 1) * D],
                                    vt[ch][:, lt * 128 : (lt + 1) * 128],
                                    pt_t[bi][ch][:],
                                    start=(ch == 0),
                                    stop=(ch == 1),
                                )
                        pc = g * G4 * DH
                        vc = g * G4 * D
                        if g % 8 < 5:
                            nc.scalar.copy(
                                vs_t[bi][:, vc : vc + G4 * D], ps[:, 0 : G4 * D]
                            )
                        else:
                            nc.vector.tensor_copy(
                                vs_t[bi][:, vc : vc + G4 * D], ps[:, 0 : G4 * D]
                            )
                        # s' = r^2 + p^2 + eps in ONE DVE pass (custom op)
                        nc.vector._custom_dve(
                            SQSUM_OP,
                            out=sp[:, pc : pc + G4 * DH],
                            in0=vs_t[bi][:, vc : vc + G4 * D : 2],
                            in1=vs_t[bi][:, vc + 1 : vc + G4 * D : 2],
                            s0=EPS,
                        )

                    # ============ PHASE 2b: sqrt group (both batches) ===========
                    # lam = sqrt(sp) in place: sp slab becomes lam
                    for c in range(NCH):
                        cs = slice(c * CHP, (c + 1) * CHP)
                        s_ins = nc.scalar.activation(
                            sp[:, cs], sp[:, cs], AF.Sqrt
                        )
                        if sqrt_first[0] is None:
                            sqrt_first[0] = s_ins
                        sqrt_last[0] = s_ins
                    if use_esc:
                        nc.vector.tensor_tensor(
                            sp[:], sp[:], esc_t[:], ALU.mult
                        )

                    # ============ PHASE 2c: trig + finish (both batches) ========
                    lam = sp
                    for c in range(NCH):
                        cs = slice(c * CHP, (c + 1) * CHP)
                        v0 = c * CHP * 2
                        v1 = (c + 1) * CHP * 2
                        rview = vs_t[bi][:, v0 : v1 : 2]
                        pview = vs_t[bi][:, v0 + 1 : v1 : 2]
                        # den = lam + r (in-place over r)
                        nc.vector.tensor_tensor(rview, lam[:, cs], rview, ALU.add)
                        # rec = 1/den (in-place over den)
                        nc.vector.reciprocal_approx_fast(out=rview, in_=rview)
                        # u = p * rec (in-place over p)
                        nc.vector.tensor_tensor(pview, pview, rview, ALU.mult)
                        a_t = pairp.tile(
                            [128, CHP], F32, tag="at", name="at", bufs=2
                        )
                        t_ins = nc.scalar.activation(a_t[:], pview, AF.Arctan)
                        if trig_first[0] is None:
                            trig_first[0] = t_ins
                        if use_delta_vec:
                            nc.vector.tensor_tensor(
                                a_t[:], a_t[:], d2_t[:, cs], ALU.mult
                            )
                            sscale = 1.0
                        else:
                            sscale = two_delta
                        oe = pairp.tile(
                            [128, CHP], F32, tag="oe", name="oe", bufs=2
                        )
                        nc.scalar.activation(
                            oe[:], a_t[:], AF.Sin, bias=HALF_PI, scale=sscale
                        )
                        oo = pairp.tile(
                            [128, CHP], F32, tag="oo", name="oo", bufs=2
                        )
                        nc.scalar.activation(
                            oo[:], a_t[:], AF.Sin, bias=0.0, scale=sscale
                        )
                        nc.vector.tensor_tensor(rview, lam[:, cs], oe[:], ALU.mult)
                        nc.vector.tensor_tensor(pview, lam[:, cs], oo[:], ALU.mult)

                    for g in range(NG4):
                        dst = out_d[bi, g * G4 * LT : (g + 1) * G4 * LT, :].rearrange(
                            "(t p) j -> p t j", p=128
                        )
                        nc.sync.dma_start(
                            dst,
                            vs_t[bi][:, g * G4 * D : (g + 1) * G4 * D].rearrange(
                                "p (t j) -> p t j", j=D
                            ),
                        )

            # ---- ACT table-set grouping edges ------------------------------
            # within a unit (rep, batch): exp -> sqrt -> trig (3 loads/unit).
            for exp_last, sqrt_first, sqrt_last, trig_first in unit_marks:
                if sqrt_first[0] is not None and exp_last[0] is not None:
                    add_dep_helper(
                        sqrt_first[0].ins, exp_last[0].ins, sync=False,
                        reason="act-set: exps before sqrts",
                    )
                if trig_first[0] is not None and sqrt_last[0] is not None:
                    add_dep_helper(
                        trig_first[0].ins, sqrt_last[0].ins, sync=False,
                        reason="act-set: sqrts before trig",
                    )
            if global_sets:
                # additionally group pairs of units: [exp exp][sqrt sqrt][trig
                # trig] -> 3 loads per 2 units. u1's sqrt waits u0's... and
                # u0's trig waits u1's sqrt-last.
                for k in range(0, len(unit_marks) - 1, 2):
                    e0, q0f, q0l, t0 = unit_marks[k]
                    e1, q1f, q1l, t1 = unit_marks[k + 1]
                    if q0f[0] is not None and e1[0] is not None:
                        add_dep_helper(
                            q0f[0].ins, e1[0].ins, sync=False,
                            reason="act-set global: u1 exp before u0 sqrt",
                        )
                    if t0[0] is not None and q1l[0] is not None:
                        add_dep_helper(
                            t0[0].ins, q1l[0].ins, sync=False,
                            reason="act-set global: u1 sqrt before u0 trig",
                        )

    nc.compile()
    return nc


_PROGRAM_CACHE = {}


def kernel(v, delta, b, log_scale):
    v = np.ascontiguousarray(v, dtype=np.float32)
    delta = np.asarray(delta, dtype=np.float32).reshape(-1)
    log_scale = np.asarray(log_scale, dtype=np.float32).reshape(-1)

    esc = np.exp(np.clip(log_scale, -5.0, 5.0)).astype(np.float32)
    use_esc = bool(np.any(esc != 1.0))
    use_delta_vec = bool(np.any(delta != delta[0]))
    two_delta = float(2.0 * delta[0])

    key = (use_esc, use_delta_vec, None if use_delta_vec else two_delta)
    if key not in _PROGRAM_CACHE:
        _PROGRAM_CACHE[key] = build_program(two_delta, use_esc, use_delta_vec)
    nc = _PROGRAM_CACHE[key]

    scaling = (D + 1 - 2 * (np.arange(D) + 1)).astype(np.float32) / (L * TAU)
    scal = np.ascontiguousarray(
        np.stack([scaling[0:128], scaling[128:256]], axis=1)
    ).astype(np.float32)

    shared = {"ident": np.eye(128, dtype=np.float32), "scalecol": scal}
    if use_esc:
        shared["esc_rep"] = np.ascontiguousarray(
            np.broadcast_to(np.tile(esc, NLT)[None, :], (128, PAIR_COLS))
        ).astype(np.float32)
    if use_delta_vec:
        shared["delta2_rep"] = np.ascontiguousarray(
            np.broadcast_to(np.tile(2.0 * delta, NLT)[None, :], (128, PAIR_COLS))
        ).astype(np.float32)

    in_maps = []
    for c in range(NCORES):
        m = dict(shared)
        m["v"] = np.ascontiguousarray(v[c * B_PER : (c + 1) * B_PER])
        in_maps.append(m)

    res = run_bass_kernel_spmd(nc, in_maps, list(range(NCORES)))
    out = np.concatenate([r["out"] for r in res.results], axis=0)
    return out.astype(np.float32)



# revision 2
# speedup vs baseline: 1.0049x; 1.0049x over previous
"""EulerFormer kernel for Trainium2 (8 NeuronCores, data-parallel over batch).

Math (per batch):
    scores = mean_l v[l, :]                          # [D]
    P      = neuralsort-softmax(scores)              # [D, D]
    vs     = v @ P.T                                 # [L, D]
    r, p   = vs[..., ::2], vs[..., 1::2]
    lam    = sqrt(r^2 + p^2 + eps)
    theta  = atan2(p, r) * delta  (== 2*atan(p/(lam+r)) * delta)
    out    = interleave(lam' * cos(theta), lam' * sin(theta)),
             lam' = lam * exp(clip(log_scale, -5, 5))

Sharding: batch 16 -> 2 per core across 8 cores. Per core:
  phase 1 (both batches): load v, PE-transpose to vT (f32r rounded on the
    PSUM->SBUF evac, which also yields scores partial sums), build P, PT.
  phase 2 (both batches): f32r einsum on PE; elementwise epilogue on ACT/DVE.
  ACT table sets are grouped program-wide: [exp (P)] -> [sqrt] -> [trig],
  enforced with scheduling-order edges, so only ~3 table loads happen.
"""

import sys

sys.path.insert(0, "/opt/trn_rl_repo")

import numpy as np

import concourse.bacc as bacc
import concourse.mybir as mybir
import concourse.tile as tile
from concourse.bass_utils import run_bass_kernel_spmd
from concourse.tile_rust import add_dep_helper

import concourse.dve_ops as dve_ops
from concourse.dve_spec import Spec, Src0, Src1, C0, sq, _has_src1, lower as dve_lower
from concourse.dve_uop import DveOpSpec


def _np_sqsum_ref(in0, in1, c0, c1, c2):
    return in0.astype(np.float32) ** 2 + in1.astype(np.float32) ** 2 + c0


def _register_sqsum():
    """out = in0^2 + in1^2 + s0, one DVE pass (replaces square+square+add)."""
    name = "TENSOR_SQSUM_ANT"
    if name in dve_ops._SUB_OPCODE_FOR_NAME:
        return next(op for op in dve_ops.OPS if op.name == name)
    spec = Spec(body=sq(Src0) + sq(Src1) + C0, reference=_np_sqsum_ref)
    row = max(dve_ops._SUB_OPCODE_FOR_NAME.values()) + 1
    assert row < 0x20
    op = dve_ops.DveOp(name, spec, subdim=False, uops_sha={})
    dve_ops._SUB_OPCODE_FOR_NAME[name] = row
    dve_ops.OPS.append(op)
    dve_ops.CUSTOM_DVE_SPECS[name] = spec
    for ver in ("v3", "v4"):
        try:
            compiled = DveOpSpec(
                name=name, opcode=row, uops=dve_lower(spec, ver=ver),
                rd1_en=_has_src1(spec),
            )
            op.uops_sha[ver] = compiled.sha(ver)
        except Exception:
            pass
    return op


SQSUM_OP = _register_sqsum()

F32 = mybir.dt.float32
F32R = mybir.dt.float32r
AF = mybir.ActivationFunctionType
ALU = mybir.AluOpType
AX = mybir.AxisListType

B, L, D = 16, 4096, 256
NCORES = 8
B_PER = B // NCORES  # 2
DH = D // 2  # 128 pairs per l-row
LT = 128
NLT = L // LT  # 32 l-tiles
TAU = 1.0
EPS = 1e-6
HALF_PI = float(np.pi / 2)

G8 = 8  # l-tiles per load/transpose group
NG8 = NLT // G8  # 4
G4 = 4  # l-tiles per einsum psum group
NG4 = NLT // G4  # 8
PAIR_COLS = NLT * DH  # 4096 pair columns per batch
VS_COLS = NLT * D  # 8192 vs-slab columns per batch
NCH = 4  # epilogue chunks per batch
CHP = PAIR_COLS // NCH  # 2048 pair cols per chunk


def _reg_consts(nc, vals):
    for val in vals:
        val = float(val)
        if (F32, val) in nc.const_aps.aps:
            continue
        t = nc.alloc_sbuf_tensor(f"const-float32-{val}", [128, 1], F32)
        nc.gpsimd.memset(t.ap(), val)
        nc.const_aps.aps[(F32, val)] = t.ap()
    nc.all_engine_barrier()


def build_program(two_delta, use_esc, use_delta_vec, reps=1, global_sets=False):
    nc = bacc.Bacc("TRN2", target_bir_lowering=False, debug=False)
    _reg_consts(nc, [HALF_PI])

    v_d = nc.dram_tensor("v", [B_PER, L, D], F32, kind="ExternalInput").ap()
    ident_d = nc.dram_tensor("ident", [128, 128], F32, kind="ExternalInput").ap()
    scal_d = nc.dram_tensor("scalecol", [128, 2], F32, kind="ExternalInput").ap()
    if use_esc:
        esc_d = nc.dram_tensor(
            "esc_rep", [128, PAIR_COLS], F32, kind="ExternalInput"
        ).ap()
    if use_delta_vec:
        d2_d = nc.dram_tensor(
            "delta2_rep", [128, PAIR_COLS], F32, kind="ExternalInput"
        ).ap()
    out_d = nc.dram_tensor("out", [B_PER, L, D], F32, kind="ExternalOutput").ap()

    with tile.TileContext(nc) as tc:
        with (
            tc.tile_pool(name="aux", bufs=1) as aux,
            tc.tile_pool(name="vload", bufs=2) as vload,
            tc.tile_pool(name="vt", bufs=1) as vtp,
            tc.tile_pool(name="vs", bufs=1) as vsp,
            tc.tile_pool(name="pair", bufs=3) as pairp,
            tc.tile_pool(name="pbuild", bufs=2) as pb,
            tc.tile_pool(name="ptr", bufs=1) as ptrp,
            tc.tile_pool(name="psT", bufs=2, space="PSUM") as psT,
            tc.tile_pool(name="psMM", bufs=2, space="PSUM") as psMM,
        ):
            ident = aux.tile([128, 128], F32, tag="ident", name="ident")
            nc.sync.dma_start(ident[:], ident_d)
            scal = aux.tile([128, 2], F32, tag="scal", name="scal")
            nc.sync.dma_start(scal[:], scal_d)
            ones_row = aux.tile([1, 128], F32, tag="ones", name="ones")
            nc.gpsimd.memset(ones_row[:], 1.0)
            if use_esc:
                esc_t = aux.tile([128, PAIR_COLS], F32, tag="esc", name="esc")
                nc.sync.dma_start(esc_t[:], esc_d)
            if use_delta_vec:
                d2_t = aux.tile([128, PAIR_COLS], F32, tag="d2", name="d2")
                nc.sync.dma_start(d2_t[:], d2_d)

            vs_t = [
                vsp.tile([128, VS_COLS], F32, tag=f"vs{bi}", name=f"vs{bi}")
                for bi in range(B_PER)
            ]
            pt_t = [
                [
                    ptrp.tile([128, D], F32R, tag=f"pt{bi}{ch}", name=f"pt{bi}{ch}")
                    for ch in range(2)
                ]
                for bi in range(B_PER)
            ]

            rowrep_n = [0]

            def rowrep(colpair_tile, scale, tagbase):
                """[128, 2] per-chunk columns -> [128, 256] row-replicated."""
                ps1 = psMM.tile([128, G4 * D], F32, tag="psmm", name="psmm")
                nc.tensor.transpose(ps1[0:1, 0:128], colpair_tile[:, 0:1], ident[:])
                nc.tensor.transpose(ps1[0:1, 128:256], colpair_tile[:, 1:2], ident[:])
                rowrep_n[0] += 1
                flat = pb.tile([1, D], F32, tag=f"{tagbase}f", name=f"{tagbase}f{rowrep_n[0]}")
                nc.scalar.activation(flat[:], ps1[0:1, 0:D], AF.Identity, scale=scale)
                ps2 = psMM.tile([128, G4 * D], F32, tag="psmm", name="psmm")
                nc.tensor.matmul(
                    ps2[:, 0:D], ones_row[:], flat[:], start=True, stop=True
                )
                rep = pb.tile([128, D], F32, tag=f"{tagbase}r", name=f"{tagbase}r{rowrep_n[0]}")
                nc.scalar.activation(rep[:], ps2[:, 0:D], AF.Identity)
                return rep

            unit_marks = []

            for rep in range(reps):
                vt_all = []
                sp_t = []
                # ===== per batch: load/transpose/scores/P then einsum+D0 ====
                for bi in range(B_PER):
                    exp_last = [None]
                    sqrt_first, sqrt_last = [None], [None]
                    trig_first = [None]
                    unit_marks.append((exp_last, sqrt_first, sqrt_last, trig_first))
                    vt = [
                        vtp.tile([128, L], F32R, tag=f"vt{ch}", name=f"vt{bi}{ch}")
                        for ch in range(2)
                    ]
                    vt_all.append(vt)
                    partials = pb.tile(
                        [128, 2 * NG8], F32, tag="part", name=f"part{bi}"
                    )
                    for g in range(NG8):
                        lv = vload.tile([128, G8 * D], F32, tag="lv", name="lv")
                        src = v_d[bi, g * G8 * LT : (g + 1) * G8 * LT, :].rearrange(
                            "(t p) j -> p t j", p=128
                        )
                        nc.sync.dma_start(
                            lv[:].rearrange("p (t j) -> p t j", j=D), src
                        )
                        for ch in range(2):
                            pst = psT.tile(
                                [128, G8 * 128], F32, tag="psT", name="psT"
                            )
                            for t in range(G8):
                                nc.tensor.transpose(
                                    pst[:, t * 128 : (t + 1) * 128],
                                    lv[:, t * D + ch * 128 : t * D + (ch + 1) * 128],
                                    ident[:],
                                )
                            dst = vt[ch][:, g * G8 * LT : (g + 1) * G8 * LT]
                            acc = partials[:, ch * NG8 + g : ch * NG8 + g + 1]
                            nc.scalar.activation(
                                dst, pst[:], AF.Identity, accum_out=acc
                            )

                    # ----- scores + P build --------------------------------
                    ssum = pb.tile([128, 2], F32, tag="ssum", name=f"ssum{bi}")
                    for ch in range(2):
                        nc.vector.tensor_reduce(
                            ssum[:, ch : ch + 1],
                            partials[:, ch * NG8 : (ch + 1) * NG8],
                            axis=AX.X,
                            op=ALU.add,
                        )
                    srow = rowrep(ssum, 1.0, "sr")
                    bsum = pb.tile([128, 2], F32, tag="bsum", name=f"bsum{bi}")
                    scratch = pb.tile([128, D], F32, tag="scr", name=f"scr{bi}")
                    for ch in range(2):
                        nc.vector.tensor_scalar(
                            scratch[:], srow[:], ssum[:, ch : ch + 1], None,
                            ALU.subtract,
                        )
                        nc.scalar.activation(
                            scratch[:], scratch[:], AF.Abs,
                            accum_out=bsum[:, ch : ch + 1],
                        )
                    brow = rowrep(bsum, 1.0 / (L * TAU), "br")

                    expp = [None, None]
                    for ch in range(2):
                        pmax = pb.tile(
                            [128, D], F32, tag=f"pmax{ch}", name=f"pmax{bi}{ch}"
                        )
                        nc.vector.scalar_tensor_tensor(
                            pmax[:], srow[:], scal[:, ch : ch + 1], brow[:],
                            ALU.mult, ALU.subtract,
                        )
                        negm = pb.tile(
                            [128, 1], F32, tag=f"negm{ch}", name=f"negm{bi}{ch}"
                        )
                        nc.vector.tensor_reduce(
                            negm[:], pmax[:], axis=AX.X, op=ALU.max, negate=True
                        )
                        rowsum = pb.tile(
                            [128, 1], F32, tag=f"rs{ch}", name=f"rs{bi}{ch}"
                        )
                        expp[ch] = pb.tile(
                            [128, D], F32, tag=f"expp{ch}", name=f"expp{bi}{ch}"
                        )
                        e_ins = nc.scalar.activation(
                            expp[ch][:], pmax[:], AF.Exp,
                            bias=negm[:], accum_out=rowsum[:],
                        )
                        exp_last[0] = e_ins
                        rinv = pb.tile(
                            [128, 1], F32, tag=f"rinv{ch}", name=f"rinv{bi}{ch}"
                        )
                        rscr = pb.tile(
                            [128, 1], F32, tag=f"rscr{ch}", name=f"rscr{bi}{ch}"
                        )
                        nc.vector.reciprocal_approx_accurate(
                            out=rinv[:], in_=rowsum[:], scratch=rscr[:]
                        )
                        nc.vector.tensor_scalar(
                            expp[ch][:], expp[ch][:], rinv[:], None, ALU.mult
                        )
                    for ech in range(2):
                        ps_pt = psMM.tile([128, G4 * D], F32, tag="psmm", name="psmm")
                        for dch in range(2):
                            nc.tensor.transpose(
                                ps_pt[:, dch * 128 : (dch + 1) * 128],
                                expp[dch][:, ech * 128 : (ech + 1) * 128],
                                ident[:],
                            )
                        nc.vector.tensor_copy(pt_t[bi][ech][:], ps_pt[:, 0:D])

                    # -------- einsum + D0 + s' (this batch) ---------------
                    sp = pairp.tile(
                        [128, PAIR_COLS], F32, tag=f"sp{bi}", name=f"sp{bi}", bufs=1
                    )
                    sp_t.append(sp)
                    for g in range(NG4):
                        ps = psMM.tile([128, G4 * D], F32, tag="psmm", name="psmm")
                        for t in range(G4):
                            lt = g * G4 + t
                            for ch in range(2):
                                nc.tensor.matmul(
                                    ps[:, t * D : (t + 1) * D],
                                    vt[ch][:, lt * 128 : (lt + 1) * 128],
                                    pt_t[bi][ch][:],
                                    start=(ch == 0),
                                    stop=(ch == 1),
                                )
                        pc = g * G4 * DH
                        vc = g * G4 * D
                        if g % 8 < 5:
                            nc.scalar.copy(
                                vs_t[bi][:, vc : vc + G4 * D], ps[:, 0 : G4 * D]
                            )
                        else:
                            nc.vector.tensor_copy(
                                vs_t[bi][:, vc : vc + G4 * D], ps[:, 0 : G4 * D]
                            )
                        # s' = r^2 + p^2 + eps in ONE DVE pass (custom op)
                        nc.vector._custom_dve(
                            SQSUM_OP,
                            out=sp[:, pc : pc + G4 * DH],
                            in0=vs_t[bi][:, vc : vc + G4 * D : 2],
                            in1=vs_t[bi][:, vc + 1 : vc + G4 * D : 2],
                            s0=EPS,
                        )

                    # ============ PHASE 2b: sqrt group (both batches) ===========
                    # lam = sqrt(sp) in place: sp slab becomes lam
                    for c in range(NCH):
                        cs = slice(c * CHP, (c + 1) * CHP)
                        s_ins = nc.scalar.activation(
                            sp[:, cs], sp[:, cs], AF.Sqrt
                        )
                        if sqrt_first[0] is None:
                            sqrt_first[0] = s_ins
                        sqrt_last[0] = s_ins
                    if use_esc:
                        nc.vector.tensor_tensor(
                            sp[:], sp[:], esc_t[:], ALU.mult
                        )

                    # ============ PHASE 2c: trig + finish (both batches) ========
                    lam = sp
                    for c in range(NCH):
                        cs = slice(c * CHP, (c + 1) * CHP)
                        v0 = c * CHP * 2
                        v1 = (c + 1) * CHP * 2
                        rview = vs_t[bi][:, v0 : v1 : 2]
                        pview = vs_t[bi][:, v0 + 1 : v1 : 2]
                        # den = lam + r (in-place over r)
                        nc.vector.tensor_tensor(rview, lam[:, cs], rview, ALU.add)
                        # rec = 1/den (in-place over den)
                        nc.vector.reciprocal_approx_fast(out=rview, in_=rview)
                        # u = p * rec (in-place over p)
                        nc.vector.tensor_tensor(pview, pview, rview, ALU.mult)
                        a_t = pairp.tile(
                            [128, CHP], F32, tag="at", name="at", bufs=2
                        )
                        t_ins = nc.scalar.activation(a_t[:], pview, AF.Arctan)
                        if trig_first[0] is None:
                            trig_first[0] = t_ins
                        if use_delta_vec:
                            nc.vector.tensor_tensor(
                                a_t[:], a_t[:], d2_t[:, cs], ALU.mult
                            )
                            sscale = 1.0
                        else:
                            sscale = two_delta
                        oe = pairp.tile(
                            [128, CHP], F32, tag="oe", name="oe", bufs=2
                        )
                        nc.scalar.activation(
                            oe[:], a_t[:], AF.Sin, bias=HALF_PI, scale=sscale
                        )
                        oo = pairp.tile(
                            [128, CHP], F32, tag="oo", name="oo", bufs=2
                        )
                        nc.scalar.activation(
                            oo[:], a_t[:], AF.Sin, bias=0.0, scale=sscale
                        )
                        nc.vector.tensor_tensor(rview, lam[:, cs], oe[:], ALU.mult)
                        nc.vector.tensor_tensor(pview, lam[:, cs], oo[:], ALU.mult)

                    for g in range(NG4):
                        dst = out_d[bi, g * G4 * LT : (g + 1) * G4 * LT, :].rearrange(
                            "(t p) j -> p t j", p=128
                        )
                        nc.sync.dma_start(
                            dst,
                            vs_t[bi][:, g * G4 * D : (g + 1) * G4 * D].rearrange(
                                "p (t j) -> p t j", j=D
                            ),
                        )

            # ---- ACT table-set grouping edges ------------------------------
            # within a unit (rep, batch): exp -> sqrt -> trig (3 loads/unit).
            for exp_last, sqrt_first, sqrt_last, trig_first in unit_marks:
                if sqrt_first[0] is not None and exp_last[0] is not None:
                    add_dep_helper(
                        sqrt_first[0].ins, exp_last[0].ins, sync=False,
                        reason="act-set: exps before sqrts",
                    )
                if trig_first[0] is not None and sqrt_last[0] is not None:
                    add_dep_helper(
                        trig_first[0].ins, sqrt_last[0].ins, sync=False,
                        reason="act-set: sqrts before trig",
                    )
            if global_sets:
                # additionally group pairs of units: [exp exp][sqrt sqrt][trig
                # trig] -> 3 loads per 2 units. u1's sqrt waits u0's... and
                # u0's trig waits u1's sqrt-last.
                for k in range(0, len(unit_marks) - 1, 2):
                    e0, q0f, q0l, t0 = unit_marks[k]
                    e1, q1f, q1l, t1 = unit_marks[k + 1]
                    if q0f[0] is not None and e1[0] is not None:
                        add_dep_helper(
                            q0f[0].ins, e1[0].ins, sync=False,
                            reason="act-set global: u1 exp before u0 sqrt",
                        )
                    if t0[0] is not None and q1l[0] is not None:
                        add_dep_helper(
                            t0[0].ins, q1l[0].ins, sync=False,
                            reason="act-set global: u1 sqrt before u0 trig",
                        )

    nc.compile()
    return nc


_PROGRAM_CACHE = {}


def _analyze_params(delta, log_scale):
    delta = np.asarray(delta, dtype=np.float32).reshape(-1)
    log_scale = np.asarray(log_scale, dtype=np.float32).reshape(-1)
    esc = np.exp(np.clip(log_scale, -5.0, 5.0)).astype(np.float32)
    use_esc = bool(np.any(esc != 1.0))
    use_delta_vec = bool(np.any(delta != delta[0]))
    two_delta = float(2.0 * delta[0])
    return delta, esc, use_esc, use_delta_vec, two_delta


def build_in_maps(inputs):
    v = np.ascontiguousarray(inputs["v"], dtype=np.float32)
    delta, esc, use_esc, use_delta_vec, _ = _analyze_params(
        inputs["delta"], inputs["log_scale"]
    )

    scaling = (D + 1 - 2 * (np.arange(D) + 1)).astype(np.float32) / (L * TAU)
    scal = np.ascontiguousarray(
        np.stack([scaling[0:128], scaling[128:256]], axis=1)
    ).astype(np.float32)

    shared = {"ident": np.eye(128, dtype=np.float32), "scalecol": scal}
    if use_esc:
        shared["esc_rep"] = np.ascontiguousarray(
            np.broadcast_to(np.tile(esc, NLT)[None, :], (128, PAIR_COLS))
        ).astype(np.float32)
    if use_delta_vec:
        shared["delta2_rep"] = np.ascontiguousarray(
            np.broadcast_to(np.tile(2.0 * delta, NLT)[None, :], (128, PAIR_COLS))
        ).astype(np.float32)

    in_maps = []
    for c in range(NCORES):
        m = dict(shared)
        m["v"] = np.ascontiguousarray(v[c * B_PER : (c + 1) * B_PER])
        in_maps.append(m)
    return in_maps


def kernel(v, delta, b, log_scale):
    _, _, use_esc, use_delta_vec, two_delta = _analyze_params(delta, log_scale)

    key = (use_esc, use_delta_vec, None if use_delta_vec else two_delta)
    if key not in _PROGRAM_CACHE:
        _PROGRAM_CACHE[key] = build_program(two_delta, use_esc, use_delta_vec)
    nc = _PROGRAM_CACHE[key]

    in_maps = build_in_maps(
        {"v": v, "delta": delta, "b": b, "log_scale": log_scale}
    )

    res = run_bass_kernel_spmd(nc, in_maps, list(range(NCORES)))
    out = np.concatenate([r["out"] for r in res.results], axis=0)
    return out.astype(np.float32)

